# revision 20
# baseline (speedup 1.0000x reference)
"""Self-contained Trainium2 (Bass/Tile) multi-head attention kernel.

Problem: nn_MultiHeadAttention (B=4, T=2048, C=1024, H=16 heads, D=64),
fp32, causal, torch-Linear-style projections (y = x @ W.T + b).

Sharding (8 NeuronCores): data-parallel over B (4) x tensor-parallel over
head-groups (2 groups of 8 heads). Core c handles batch c//2, head group
c%2. Each core computes Q/K/V projections for its 512 features, causal
attention for its 8 heads, and a partial output projection
(O_group @ Wo[:, group].T). The host sums the two partials per batch and
adds bo.

On-device layout (per core) is fully "transposed" so that no tensor ever
needs an on-chip transpose:
  - Q^T, K^T: [feat 512, tok T]   (feature-major; head pair per 128-chunk)
  - V_aug:    [tok T, 8*65]       (token-major; per head 64 V cols + ones col)
  - S^T tile: [k 128, q 512] = matmul(lhsT=K^T chunk, rhs=Q^T chunk), K=64
  - P^T = exp(0.125 * S^T + mask) (ScalarE, bf16 out)
  - PV: [65, q 512] += matmul(lhsT=V_aug[k-tile, head], rhs=P^T) over k
        rows 0..63 = unnormalized O^T, row 64 = softmax denominators
  - normalize: r = 1/sums (DVE), broadcast across partitions with a K=1
    matmul (ones^T @ r), multiply on DVE -> O^T [feat, tok] bf16
  - out partial [tok, C] = matmul(lhsT=O^T chunks, rhs=Wo_g^T chunks)
"""

import numpy as np
import ml_dtypes

import bass_rust
import concourse.bass as bass
import concourse.mybir as mybir
import concourse.tile as tile
from concourse.bass_utils import run_bass_kernel_spmd
from concourse.vector_clock import ScopedClock

BF16 = ml_dtypes.bfloat16

B, T, C, H, D = 4, 2048, 1024, 16, 64
G = C // 2          # features per head group (8 heads x 64)
N_CORES = 8
MASK_NEG = -800.0   # pre-scale; exp(0.125 * (s - 800)) == 0 for |s| < 30

# ---------------------------------------------------------------------------
# The walrus build in this container rejects instructions carrying more than
# a couple of sync waits ("Too many sync wait commands"). Tile's kernel-tail
# drain aggregates one wait per live semaphore. Split them into individual
# SP wait instructions (program order on SP preserves the semantics).
# ---------------------------------------------------------------------------


def _patched_drain_and_barrier(self, tick_clock, wait_clock):
    nc = self.nc
    drain_inst = nc.sync.drain()
    wait_clock.add_sem_waits(
        drain_inst.ins, ScopedClock({None: tick_clock.global_clock})
    )
    si = drain_inst.ins.sync_info
    waits = list(si.on_wait) if si is not None else []
    if waits:
        drain_inst.ins.sync_info = bass_rust.SyncInfo(
            on_wait=[], on_update=list(si.on_update)
        )
        assert self.sems is not None
        by_name = {h.name: h for h in self.sems.allocated().values()}
        for w in waits:
            assert w.wait_mode == "sem-ge-imm", w
            nc.sync.wait_ge(by_name[w.ant_name], w.wait_value)

    nc.all_engine_barrier()
    assert self.sems is not None
    popped = nc._tile_sem_poison_stack.pop()
    assert popped is self._sem_poison
    nc.clear_and_free_semaphores(list(self.sems.allocated().values()))
    nc.all_engine_barrier()


tile.TileContext._drain_and_barrier = _patched_drain_and_barrier


def _split_excess_waits(nc, max_waits=1):
    """Hoist surplus sync waits into standalone same-engine EventSemaphore
    instructions placed right before the owner (this walrus encodes at most
    one wait per instruction)."""
    n = 0
    for fn in nc.m.functions:
        for blk in fn.blocks:
            new_insts = []
            for inst in blk.instructions:
                si = inst.sync_info
                waits = list(si.on_wait) if si is not None else []
                if len(waits) > max_waits:
                    for w in waits[:-max_waits]:
                        ev = mybir.InstEventSemaphore(
                            name=f"I-wsplit-{n}", ins=[], outs=[]
                        )
                        n += 1
                        ev.engine = inst.engine
                        ev.sync_info = bass_rust.SyncInfo(
                            on_wait=[w], on_update=[]
                        )
                        new_insts.append(ev)
                    inst.sync_info = bass_rust.SyncInfo(
                        on_wait=waits[-max_waits:], on_update=list(si.on_update)
                    )
                new_insts.append(inst)
            blk.instructions = new_insts


# ---------------------------------------------------------------------------
# Kernel builder (per-core program; same program on all 8 cores)
# ---------------------------------------------------------------------------

def build_nc(t=T, split_waits=True):
    f32 = mybir.dt.float32
    f32r = mybir.dt.float32r
    bf16 = mybir.dt.bfloat16
    Exp = mybir.ActivationFunctionType.Exp

    assert t % 512 == 0
    TS = t // 512            # 512-token slices (q-tiles)
    TK = t // 128            # 128-token k-tiles

    nc = bass.Bass()
    xt_d = nc.dram_tensor("xt", [C, t], bf16, kind="ExternalInput")
    wqt_d = nc.dram_tensor("wqt", [C, G], bf16, kind="ExternalInput")
    wkt_d = nc.dram_tensor("wkt", [C, G], bf16, kind="ExternalInput")
    wvt_d = nc.dram_tensor("wvt", [C, G], bf16, kind="ExternalInput")
    wot_d = nc.dram_tensor("wot", [G, C], bf16, kind="ExternalInput")
    bqk_d = nc.dram_tensor("bqk", [128, 8], f32, kind="ExternalInput")
    bv_d = nc.dram_tensor("bv", [G], f32, kind="ExternalInput")
    mask_d = nc.dram_tensor("mask", [128, 512], f32, kind="ExternalInput")
    out_d = nc.dram_tensor("out", [t, C], f32, kind="ExternalOutput")
    rsc_d = nc.dram_tensor("rscratch", [4, 512], f32, kind="ExternalOutput")

    with tile.TileContext(nc) as tc:
        with (
            tc.tile_pool(name="big", bufs=1) as big,
            tc.tile_pool(name="weights", bufs=1) as wpool,
            tc.tile_pool(name="xsl", bufs=3) as xpool,
            tc.tile_pool(name="pt", bufs=6) as ptpool,
            tc.tile_pool(name="small", bufs=4) as small,
            tc.tile_pool(name="psA", bufs=2, space="PSUM") as psA,
            tc.tile_pool(name="psS", bufs=2, space="PSUM") as psS,
            tc.tile_pool(name="psPV", bufs=2, space="PSUM") as psPV,
        ):
            # ---- persistent SBUF tensors ----
            qt_sb = big.tile([128, 4, t], bf16, tag="qt")      # Q^T
            kt_sb = big.tile([128, 4, t], bf16, tag="kt")      # K^T
            vaug = big.tile([128, TK, 8 * 65], bf16, tag="va")  # V + ones col
            ot_sb = big.tile([128, 4, t], bf16, tag="ot")      # O^T normalized

            bv_sb = wpool.tile([128, G], f32, tag="bv")
            nc.sync.dma_start(
                out=bv_sb, in_=bv_d[:].unsqueeze(0).to_broadcast((128, G))
            )
            w_sb = {}
            for name, d_t in (("v", wvt_d), ("q", wqt_d), ("k", wkt_d)):
                w = wpool.tile([128, 8, G], bf16, tag=f"w{name}")
                wsrc = d_t[:, :].rearrange("(c p) f -> p c f", p=128)
                nc.sync.dma_start(out=w[:, 0:4, :], in_=wsrc[:, 0:4, :])
                nc.sync.dma_start(out=w[:, 4:8, :], in_=wsrc[:, 4:8, :])
                w_sb[name] = w
            mask_sb = wpool.tile([128, 512], f32, tag="mask")
            nc.sync.dma_start(out=mask_sb, in_=mask_d[:, :])
            bqk_sb = wpool.tile([128, 8], f32, tag="bqk")
            nc.sync.dma_start(out=bqk_sb, in_=bqk_d[:, :])
            ones_sb = wpool.tile([1, 64], bf16, tag="ones")
            nc.vector.memset(ones_sb, 1.0)
            # ones columns of V_aug (head h, column 64)
            va_ones = vaug[:].rearrange("p k (h x) -> p k h x", x=65)[:, :, :, 64]
            nc.vector.memset(va_ones, 1.0)
            wot_sb = wpool.tile([128, 4, C], bf16, tag="wot")
            nc.sync.dma_start(
                out=wot_sb, in_=wot_d[:, :].rearrange("(c p) f -> p c f", p=128)
            )

            def load_x_slice(ts):
                tsl = slice(ts * 512, (ts + 1) * 512)
                x_sl = xpool.tile([128, 8, 512], bf16, tag="xsl")
                src = xt_d[:, tsl].rearrange("(c p) t -> p c t", p=128)
                nc.sync.dma_start(out=x_sl[:, 0:4, :], in_=src[:, 0:4, :])
                nc.sync.dma_start(out=x_sl[:, 4:8, :], in_=src[:, 4:8, :])
                return x_sl

            def v_group(ts):
                """V projection for 4 token sub-tiles: [tok 128, feat 512]
                scattered into vaug (stride 65, per-head ones column kept)."""
                x_sl = load_x_slice(ts)
                for tsub in range(4):
                    kt_idx = ts * 4 + tsub
                    ps = psA.tile([128, 512], f32, tag="mm")
                    for cc in range(8):
                        nc.tensor.matmul(
                            ps,
                            lhsT=x_sl[:, cc, tsub * 128:(tsub + 1) * 128],
                            rhs=w_sb["v"][:, cc, :],
                            start=(cc == 0),
                            stop=(cc == 7),
                        )
                    dst = vaug[:, kt_idx, :].rearrange(
                        "p (h x) -> p h x", x=65)[:, :, 0:64]
                    nc.vector.tensor_add(
                        out=dst,
                        in0=ps.rearrange("p (h d) -> p h d", d=64),
                        in1=bv_sb.rearrange("p (h d) -> p h d", d=64),
                    )

            def qk_group(c, ts):
                """Q^T and K^T projection tiles [feat 128, tok 512] for
                head-pair chunk c, token slice ts."""
                tsl = slice(ts * 512, (ts + 1) * 512)
                x_sl = load_x_slice(ts)
                for name, dst, bcol in (("q", qt_sb, 0), ("k", kt_sb, 4)):
                    ps = psA.tile([128, 512], f32, tag="mm")
                    for cc in range(8):
                        nc.tensor.matmul(
                            ps,
                            lhsT=w_sb[name][:, cc, c * 128:(c + 1) * 128],
                            rhs=x_sl[:, cc, :],
                            start=(cc == 0),
                            stop=(cc == 7),
                        )
                    # copy + per-feature bias (exact, on DVE)
                    with nc.allow_low_precision(
                        reason="Q^T/K^T stored as fp32r for the PE"
                    ):
                        nc.vector.tensor_scalar_add(
                            out=dst[:, c, tsl],
                            in0=ps,
                            scalar1=bqk_sb[:, bcol + c:bcol + c + 1],
                        )

            def outproj_group(tt, of):
                """Partial output projection [tok 128, outfeat 512]."""
                ps = psA.tile([128, 512], f32, tag="mm")
                for fc in range(4):
                    nc.tensor.matmul(
                        ps,
                        lhsT=ot_sb[:, fc, tt * 128:(tt + 1) * 128],
                        rhs=wot_sb[:, fc, of * 512:(of + 1) * 512],
                        start=(fc == 0),
                        stop=(fc == 3),
                    )
                o_out = small.tile([128, 512], f32, tag="oout")
                nc.vector.tensor_copy(out=o_out, in_=ps)
                nc.sync.dma_start(
                    out=out_d[tt * 128:(tt + 1) * 128,
                              of * 512:(of + 1) * 512],
                    in_=o_out,
                )

            def epilogue(c, qt, pv):
                """Normalize O^T rows of (chunk c, q-tile qt) by the softmax
                denominators in pv row 64: r = exp(-ln(sums)) on ScalarE,
                broadcast across 64 partitions by a DRAM-bounce DMA (free on
                the idle DMA queues; no custom ISA ops in this walrus), then
                one DVE multiply straight out of PSUM."""
                for hp in range(2):
                    po = hp * 64
                    lnr = small.tile([1, 512], f32, tag="lnr")
                    nc.scalar.activation(
                        lnr, pv[hp][64:65, :],
                        func=mybir.ActivationFunctionType.Ln,
                    )
                    r_sb = small.tile([1, 512], f32, tag="r")
                    nc.scalar.activation(r_sb, lnr, func=Exp, scale=-1.0)
                    slot = (qt * 2 + hp) % 4
                    nc.sync.dma_start(
                        out=rsc_d[slot:slot + 1, :], in_=r_sb[0:1, :])
                    bcast = small.tile([64, 512], f32, tag="bcast")
                    nc.sync.dma_start(
                        out=bcast,
                        in_=rsc_d[slot, :].unsqueeze(0).to_broadcast((64, 512)),
                    )
                    nc.vector.tensor_mul(
                        out=ot_sb[po:po + 64, c, qt * 512:(qt + 1) * 512],
                        in0=pv[hp][0:64, :],
                        in1=bcast,
                    )

            # ---- prologue: just enough for (chunk 0, q-tile 0) ----
            v_group(0)
            qk_group(0, 0)

            # ---- attention, with PE filler work interleaved ----
            # Fillers keep TensorE continuously busy through the ACT-bound
            # attention stretches (HAM re-throttles the PE clock to 1.2 GHz
            # after ~3.4us of sub-full activity, halving matmul speed).
            total_iters = sum(4 * (qt + 1) for qt in range(TS))
            for c in range(4):
                # fillers: (deadline_qt, emit_fn). A filler with deadline d
                # MUST be emitted before q-tile d starts (Tile dependencies
                # follow trace order, so a PV reading vaug must come after
                # the V write in emission order). Each chunk carries its own
                # later Q/K slices (deadline = their q-tile) plus the next
                # chunk's first slice, so projections overlap attention
                # maximally.
                fillers = []
                for ts in range(1, TS):
                    if c == 0:
                        fillers.append((ts, lambda ts=ts: v_group(ts)))
                    fillers.append((ts, lambda ts=ts: qk_group(c, ts)))
                if c < 3:
                    fillers.append(
                        (None, lambda cn=c + 1: qk_group(cn, 0)))
                ready_fill = list(fillers)
                spacing = 2 if c == 3 else max(
                    1, total_iters // max(1, len(fillers) + 1))
                it = 0
                for qt in range(TS):
                    while ready_fill and ready_fill[0][0] is not None \
                            and ready_fill[0][0] <= qt:
                        ready_fill.pop(0)[1]()
                    pv = [
                        psPV.tile([65, 512], f32, tag="pv", name=f"pv{i}")
                        for i in range(2)
                    ]
                    nkt = 4 * (qt + 1)
                    for kt in range(nkt):
                        j = kt - 4 * qt      # >=0 on the diagonal band
                        qoff = max(j, 0) * 128      # first valid q column
                        w = 512 - qoff
                        qsl = slice(qt * 512 + qoff, (qt + 1) * 512)
                        # both heads share one 2-bank PSUM tile so the mask
                        # add and the exp run as ONE op each (~250ns fixed
                        # cost per ACT/DVE op otherwise doubles up)
                        s_ps = psS.tile([128, 1024], f32, tag="s")
                        for hp in range(2):
                            po = hp * 64
                            nc.tensor.matmul(
                                s_ps[:, hp * 512 + qoff:(hp + 1) * 512],
                                lhsT=kt_sb[po:po + 64, c,
                                           kt * 128:(kt + 1) * 128],
                                rhs=qt_sb[po:po + 64, c, qsl],
                                start=True,
                                stop=True,
                            )
                        p_t = ptpool.tile([128, 1024], bf16, tag="pt")
                        if j >= 0:
                            s_stage = small.tile([128, 1024], f32, tag="sst")
                            sps_v = s_ps.rearrange(
                                "p (h q) -> p h q", h=2)[:, :, qoff:]
                            sst_v = s_stage.rearrange(
                                "p (h q) -> p h q", h=2)[:, :, :w]
                            nc.vector.tensor_add(
                                out=sst_v,
                                in0=sps_v,
                                in1=mask_sb[:, :w].unsqueeze(1).to_broadcast(
                                    (128, 2, w)),
                            )
                            nc.scalar.activation(
                                out=p_t.rearrange(
                                    "p (h q) -> p h q", h=2)[:, :, qoff:],
                                in_=sst_v,
                                func=Exp,
                                scale=0.125,
                            )
                        else:
                            nc.scalar.activation(
                                out=p_t, in_=s_ps, func=Exp, scale=0.125,
                            )
                        for hp in range(2):
                            h = 2 * c + hp
                            nc.tensor.matmul(
                                pv[hp][:, qoff:],
                                lhsT=vaug[:, kt, h * 65:(h + 1) * 65],
                                rhs=p_t[:, hp * 512 + qoff:(hp + 1) * 512],
                                start=(kt == 0),
                                stop=(kt == nkt - 1),
                            )
                        it += 1
                        if ready_fill and it % spacing == 0:
                            ready_fill.pop(0)[1]()
                    # a filler between the last PV and the epilogue hides the
                    # ln/exp latency from the PE's broadcast matmul
                    if ready_fill:
                        ready_fill.pop(0)[1]()
                    epilogue(c, qt, pv)
                    if c == 3:
                        for tt in range(qt * 4, qt * 4 + 4):
                            for of in range(2):
                                ready_fill.append(
                                    (None, lambda tt=tt, of=of:
                                     outproj_group(tt, of)))
                # chunk tail: remaining fillers
                for _, f in ready_fill:
                    f()

    if split_waits:
        _split_excess_waits(nc)
    return nc


# ---------------------------------------------------------------------------
# Host side
# ---------------------------------------------------------------------------

_NC_CACHE = {}


def _get_nc(t=T):
    if t not in _NC_CACHE:
        _NC_CACHE[t] = build_nc(t)
    return _NC_CACHE[t]


def make_mask():
    # [tri(128x128) | zeros(128x384)]: band tile at column offset qoff adds
    # the triangular block against S columns qoff:qoff+128 and zero beyond
    k = np.arange(128)[:, None]
    q = np.arange(128)[None, :]
    tri = np.where(k <= q, 0.0, MASK_NEG).astype(np.float32)
    return np.concatenate([tri, np.zeros((128, 384), np.float32)], axis=1)


def core_inputs(x, Wq, bq, Wk, bk, Wv, bv, Wo, core):
    """Build the input map for one core (batch b, head group g)."""
    b, g = divmod(core, 2)
    gs = slice(g * G, (g + 1) * G)
    xt = np.ascontiguousarray(x[b].T).astype(BF16)            # [C, T]
    wqt = np.ascontiguousarray(Wq[gs, :].T).astype(BF16)      # [C, G]
    wkt = np.ascontiguousarray(Wk[gs, :].T).astype(BF16)
    wvt = np.ascontiguousarray(Wv[gs, :].T).astype(BF16)
    wot = np.ascontiguousarray(Wo[:, gs].T).astype(BF16)      # [G, C]
    bqk = np.concatenate(
        [bq[gs].reshape(4, 128).T, bk[gs].reshape(4, 128).T], axis=1
    ).astype(np.float32)                                      # [128, 8]
    return {
        "xt": xt, "wqt": wqt, "wkt": wkt, "wvt": wvt, "wot": wot,
        "bqk": bqk, "bv": bv[gs].astype(np.float32), "mask": make_mask(),
    }


def kernel(x, Wq, bq, Wk, bk, Wv, bv, Wo, bo, _trace=False):
    x = np.asarray(x, dtype=np.float32)
    nc = _get_nc(T)
    in_maps = [
        core_inputs(x, Wq, bq, Wk, bk, Wv, bv, Wo, c) for c in range(N_CORES)
    ]
    res = run_bass_kernel_spmd(nc, in_maps, list(range(N_CORES)), trace=_trace)
    out = np.empty((B, T, C), dtype=np.float32)
    bo = np.asarray(bo, dtype=np.float32)
    for b in range(B):
        out[b] = res.results[2 * b]["out"] + res.results[2 * b + 1]["out"]
        out[b] += bo[None, :]
    kernel.last_results = res
    return out


# revision 21
# speedup vs baseline: 1.1148x; 1.1148x over previous
"""Self-contained Trainium2 (Bass/Tile) multi-head attention kernel.

Problem: nn_MultiHeadAttention (B=4, T=2048, C=1024, H=16 heads, D=64),
fp32, causal, torch-Linear-style projections (y = x @ W.T + b).

Sharding (8 NeuronCores): data-parallel over B (4) x tensor-parallel over
head-groups (2 groups of 8 heads). Core c handles batch c//2, head group
c%2. Each core computes Q/K/V projections for its 512 features, causal
attention for its 8 heads, and a partial output projection
(O_group @ Wo[:, group].T). The host sums the two partials per batch and
adds bo.

On-device layout (per core) is fully "transposed" so that no tensor ever
needs an on-chip transpose:
  - Q^T, K^T: [feat 512, tok T]   (feature-major; head pair per 128-chunk)
  - V_aug:    [tok T, 8*65]       (token-major; per head 64 V cols + ones col)
  - S^T tile: [k 128, q 512] = matmul(lhsT=K^T chunk, rhs=Q^T chunk), K=64
  - P^T = exp(0.125 * S^T + mask) (ScalarE, bf16 out)
  - PV: [65, q 512] += matmul(lhsT=V_aug[k-tile, head], rhs=P^T) over k
        rows 0..63 = unnormalized O^T, row 64 = softmax denominators
  - normalize: r = 1/sums (DVE), broadcast across partitions with a K=1
    matmul (ones^T @ r), multiply on DVE -> O^T [feat, tok] bf16
  - out partial [tok, C] = matmul(lhsT=O^T chunks, rhs=Wo_g^T chunks)
"""

import numpy as np
import ml_dtypes

import bass_rust
import concourse.bass as bass
import concourse.mybir as mybir
import concourse.tile as tile
from concourse.bass_utils import run_bass_kernel_spmd
from concourse.vector_clock import ScopedClock

BF16 = ml_dtypes.bfloat16

B, T, C, H, D = 4, 2048, 1024, 16, 64
G = C // 2          # features per head group (8 heads x 64)
N_CORES = 8
MASK_NEG = -800.0   # pre-scale; exp(0.125 * (s - 800)) == 0 for |s| < 30

# ---------------------------------------------------------------------------
# The walrus build in this container rejects instructions carrying more than
# a couple of sync waits ("Too many sync wait commands"). Tile's kernel-tail
# drain aggregates one wait per live semaphore. Split them into individual
# SP wait instructions (program order on SP preserves the semantics).
# ---------------------------------------------------------------------------


def _patched_drain_and_barrier(self, tick_clock, wait_clock):
    nc = self.nc
    drain_inst = nc.sync.drain()
    wait_clock.add_sem_waits(
        drain_inst.ins, ScopedClock({None: tick_clock.global_clock})
    )
    si = drain_inst.ins.sync_info
    waits = list(si.on_wait) if si is not None else []
    if waits:
        drain_inst.ins.sync_info = bass_rust.SyncInfo(
            on_wait=[], on_update=list(si.on_update)
        )
        assert self.sems is not None
        by_name = {h.name: h for h in self.sems.allocated().values()}
        for w in waits:
            assert w.wait_mode == "sem-ge-imm", w
            nc.sync.wait_ge(by_name[w.ant_name], w.wait_value)

    nc.all_engine_barrier()
    assert self.sems is not None
    popped = nc._tile_sem_poison_stack.pop()
    assert popped is self._sem_poison
    nc.clear_and_free_semaphores(list(self.sems.allocated().values()))
    nc.all_engine_barrier()


tile.TileContext._drain_and_barrier = _patched_drain_and_barrier


def _split_excess_waits(nc, max_waits=1):
    """Hoist surplus sync waits into standalone same-engine EventSemaphore
    instructions placed right before the owner (this walrus encodes at most
    one wait per instruction)."""
    n = 0
    for fn in nc.m.functions:
        for blk in fn.blocks:
            new_insts = []
            for inst in blk.instructions:
                si = inst.sync_info
                waits = list(si.on_wait) if si is not None else []
                if len(waits) > max_waits:
                    for w in waits[:-max_waits]:
                        ev = mybir.InstEventSemaphore(
                            name=f"I-wsplit-{n}", ins=[], outs=[]
                        )
                        n += 1
                        ev.engine = inst.engine
                        ev.sync_info = bass_rust.SyncInfo(
                            on_wait=[w], on_update=[]
                        )
                        new_insts.append(ev)
                    inst.sync_info = bass_rust.SyncInfo(
                        on_wait=waits[-max_waits:], on_update=list(si.on_update)
                    )
                new_insts.append(inst)
            blk.instructions = new_insts


# ---------------------------------------------------------------------------
# Kernel builder (per-core program; same program on all 8 cores)
# ---------------------------------------------------------------------------

def build_nc(t=T, split_waits=True):
    f32 = mybir.dt.float32
    f32r = mybir.dt.float32r
    bf16 = mybir.dt.bfloat16
    Exp = mybir.ActivationFunctionType.Exp

    assert t % 512 == 0
    TS = t // 512            # 512-token slices (q-tiles)
    TK = t // 128            # 128-token k-tiles

    nc = bass.Bass()
    xt_d = nc.dram_tensor("xt", [C, t], bf16, kind="ExternalInput")
    wqt_d = nc.dram_tensor("wqt", [C, G], bf16, kind="ExternalInput")
    wkt_d = nc.dram_tensor("wkt", [C, G], bf16, kind="ExternalInput")
    wvt_d = nc.dram_tensor("wvt", [C, G], bf16, kind="ExternalInput")
    wot_d = nc.dram_tensor("wot", [G, C], bf16, kind="ExternalInput")
    bqk_d = nc.dram_tensor("bqk", [128, 8], f32, kind="ExternalInput")
    bv_d = nc.dram_tensor("bv", [G], f32, kind="ExternalInput")
    mask_d = nc.dram_tensor("mask", [128, 512], f32, kind="ExternalInput")
    out_d = nc.dram_tensor("out", [t, C], f32, kind="ExternalOutput")

    with tile.TileContext(nc) as tc:
        with (
            tc.tile_pool(name="big", bufs=1) as big,
            tc.tile_pool(name="weights", bufs=1) as wpool,
            tc.tile_pool(name="xsl", bufs=3) as xpool,
            tc.tile_pool(name="pt", bufs=6) as ptpool,
            tc.tile_pool(name="small", bufs=4) as small,
            tc.tile_pool(name="psA", bufs=2, space="PSUM") as psA,
            tc.tile_pool(name="psS", bufs=2, space="PSUM") as psS,
            tc.tile_pool(name="psPV", bufs=2, space="PSUM") as psPV,
        ):
            # ---- persistent SBUF tensors ----
            qt_sb = big.tile([128, 4, t], bf16, tag="qt")      # Q^T
            kt_sb = big.tile([128, 4, t], bf16, tag="kt")      # K^T
            vaug = big.tile([128, TK, 8 * 65], bf16, tag="va")  # V + ones col
            ot_sb = big.tile([128, 4, t], bf16, tag="ot")      # O^T normalized

            bv_sb = wpool.tile([128, G], f32, tag="bv")
            nc.sync.dma_start(
                out=bv_sb, in_=bv_d[:].unsqueeze(0).to_broadcast((128, G))
            )
            w_sb = {}
            for name, d_t in (("v", wvt_d), ("q", wqt_d), ("k", wkt_d)):
                w = wpool.tile([128, 8, G], bf16, tag=f"w{name}")
                wsrc = d_t[:, :].rearrange("(c p) f -> p c f", p=128)
                nc.sync.dma_start(out=w[:, 0:4, :], in_=wsrc[:, 0:4, :])
                nc.sync.dma_start(out=w[:, 4:8, :], in_=wsrc[:, 4:8, :])
                w_sb[name] = w
            mask_sb = wpool.tile([128, 512], f32, tag="mask")
            nc.sync.dma_start(out=mask_sb, in_=mask_d[:, :])
            bqk_sb = wpool.tile([128, 8], f32, tag="bqk")
            nc.sync.dma_start(out=bqk_sb, in_=bqk_d[:, :])
            ones_sb = wpool.tile([1, 64], bf16, tag="ones")
            nc.vector.memset(ones_sb, 1.0)
            # ones columns of V_aug (head h, column 64)
            va_ones = vaug[:].rearrange("p k (h x) -> p k h x", x=65)[:, :, :, 64]
            nc.vector.memset(va_ones, 1.0)
            wot_sb = wpool.tile([128, 4, C], bf16, tag="wot")
            nc.sync.dma_start(
                out=wot_sb, in_=wot_d[:, :].rearrange("(c p) f -> p c f", p=128)
            )

            def load_x_slice(ts):
                tsl = slice(ts * 512, (ts + 1) * 512)
                x_sl = xpool.tile([128, 8, 512], bf16, tag="xsl")
                src = xt_d[:, tsl].rearrange("(c p) t -> p c t", p=128)
                nc.sync.dma_start(out=x_sl[:, 0:4, :], in_=src[:, 0:4, :])
                nc.sync.dma_start(out=x_sl[:, 4:8, :], in_=src[:, 4:8, :])
                return x_sl

            def v_group(ts):
                """V projection for 4 token sub-tiles: [tok 128, feat 512]
                scattered into vaug (stride 65, per-head ones column kept)."""
                x_sl = load_x_slice(ts)
                for tsub in range(4):
                    kt_idx = ts * 4 + tsub
                    ps = psA.tile([128, 512], f32, tag="mm")
                    for cc in range(8):
                        nc.tensor.matmul(
                            ps,
                            lhsT=x_sl[:, cc, tsub * 128:(tsub + 1) * 128],
                            rhs=w_sb["v"][:, cc, :],
                            start=(cc == 0),
                            stop=(cc == 7),
                        )
                    dst = vaug[:, kt_idx, :].rearrange(
                        "p (h x) -> p h x", x=65)[:, :, 0:64]
                    nc.vector.tensor_add(
                        out=dst,
                        in0=ps.rearrange("p (h d) -> p h d", d=64),
                        in1=bv_sb.rearrange("p (h d) -> p h d", d=64),
                    )

            def qk_group(c, ts):
                """Q^T and K^T projection tiles [feat 128, tok 512] for
                head-pair chunk c, token slice ts."""
                tsl = slice(ts * 512, (ts + 1) * 512)
                x_sl = load_x_slice(ts)
                for name, dst, bcol in (("q", qt_sb, 0), ("k", kt_sb, 4)):
                    ps = psA.tile([128, 512], f32, tag="mm")
                    for cc in range(8):
                        nc.tensor.matmul(
                            ps,
                            lhsT=w_sb[name][:, cc, c * 128:(c + 1) * 128],
                            rhs=x_sl[:, cc, :],
                            start=(cc == 0),
                            stop=(cc == 7),
                        )
                    # copy + per-feature bias (exact, on DVE)
                    with nc.allow_low_precision(
                        reason="Q^T/K^T stored as fp32r for the PE"
                    ):
                        nc.vector.tensor_scalar_add(
                            out=dst[:, c, tsl],
                            in0=ps,
                            scalar1=bqk_sb[:, bcol + c:bcol + c + 1],
                        )

            def outproj_group(tt, of):
                """Partial output projection [tok 128, outfeat 512]."""
                ps = psA.tile([128, 512], f32, tag="mm")
                for fc in range(4):
                    nc.tensor.matmul(
                        ps,
                        lhsT=ot_sb[:, fc, tt * 128:(tt + 1) * 128],
                        rhs=wot_sb[:, fc, of * 512:(of + 1) * 512],
                        start=(fc == 0),
                        stop=(fc == 3),
                    )
                o_out = small.tile([128, 512], f32, tag="oout")
                nc.vector.tensor_copy(out=o_out, in_=ps)
                nc.sync.dma_start(
                    out=out_d[tt * 128:(tt + 1) * 128,
                              of * 512:(of + 1) * 512],
                    in_=o_out,
                )

            def epilogue(c, qt, pv):
                """Normalize O^T rows of (chunk c, q-tile qt) by the softmax
                denominators in pv row 64: r = exp(-ln(sums)) on ScalarE,
                partition-broadcast with a K=1 matmul into rows 0:64 of the
                pv PSUM bank (reused after the o_stage copy), one DVE mul."""
                for hp in range(2):
                    po = hp * 64
                    lnr = small.tile([1, 512], f32, tag="lnr")
                    nc.scalar.activation(
                        lnr, pv[hp][64:65, :],
                        func=mybir.ActivationFunctionType.Ln,
                    )
                    r_sb = small.tile([1, 512], bf16, tag="r")
                    nc.scalar.activation(r_sb, lnr, func=Exp, scale=-1.0)
                    o_stage = small.tile([64, 512], f32, tag="ost")
                    nc.vector.tensor_copy(out=o_stage, in_=pv[hp][0:64, :])
                    bc_ps = pv[hp][0:64, :]
                    nc.tensor.matmul(
                        bc_ps, lhsT=ones_sb, rhs=r_sb, start=True, stop=True,
                    )
                    nc.vector.tensor_mul(
                        out=ot_sb[po:po + 64, c, qt * 512:(qt + 1) * 512],
                        in0=bc_ps,
                        in1=o_stage,
                    )

            # ---- prologue: just enough for (chunk 0, q-tile 0) ----
            v_group(0)
            qk_group(0, 0)

            # ---- attention, with PE filler work interleaved ----
            # Fillers keep TensorE continuously busy through the ACT-bound
            # attention stretches (HAM re-throttles the PE clock to 1.2 GHz
            # after ~3.4us of sub-full activity, halving matmul speed).
            total_iters = sum(4 * (qt + 1) for qt in range(TS))
            for c in range(4):
                # fillers: (deadline_qt, emit_fn). A filler with deadline d
                # MUST be emitted before q-tile d starts (Tile dependencies
                # follow trace order, so a PV reading vaug must come after
                # the V write in emission order). Each chunk carries its own
                # later Q/K slices (deadline = their q-tile) plus the next
                # chunk's first slice, so projections overlap attention
                # maximally.
                fillers = []
                for ts in range(1, TS):
                    if c == 0:
                        fillers.append((ts, lambda ts=ts: v_group(ts)))
                    fillers.append((ts, lambda ts=ts: qk_group(c, ts)))
                if c < 3:
                    fillers.append(
                        (None, lambda cn=c + 1: qk_group(cn, 0)))
                ready_fill = list(fillers)
                spacing = 2 if c == 3 else max(
                    1, total_iters // max(1, len(fillers) + 1))
                it = 0
                for qt in range(TS):
                    while ready_fill and ready_fill[0][0] is not None \
                            and ready_fill[0][0] <= qt:
                        ready_fill.pop(0)[1]()
                    pv = [
                        psPV.tile([65, 512], f32, tag="pv", name=f"pv{i}")
                        for i in range(2)
                    ]
                    nkt = 4 * (qt + 1)
                    for kt in range(nkt):
                        j = kt - 4 * qt      # >=0 on the diagonal band
                        qoff = max(j, 0) * 128      # first valid q column
                        w = 512 - qoff
                        qsl = slice(qt * 512 + qoff, (qt + 1) * 512)
                        # both heads share one 2-bank PSUM tile so the mask
                        # add and the exp run as ONE op each (~250ns fixed
                        # cost per ACT/DVE op otherwise doubles up)
                        s_ps = psS.tile([128, 1024], f32, tag="s")
                        for hp in range(2):
                            po = hp * 64
                            nc.tensor.matmul(
                                s_ps[:, hp * 512 + qoff:(hp + 1) * 512],
                                lhsT=kt_sb[po:po + 64, c,
                                           kt * 128:(kt + 1) * 128],
                                rhs=qt_sb[po:po + 64, c, qsl],
                                start=True,
                                stop=True,
                            )
                        p_t = ptpool.tile([128, 1024], bf16, tag="pt")
                        if j >= 0:
                            s_stage = small.tile([128, 1024], f32, tag="sst")
                            sps_v = s_ps.rearrange(
                                "p (h q) -> p h q", h=2)[:, :, qoff:]
                            sst_v = s_stage.rearrange(
                                "p (h q) -> p h q", h=2)[:, :, :w]
                            nc.vector.tensor_add(
                                out=sst_v,
                                in0=sps_v,
                                in1=mask_sb[:, :w].unsqueeze(1).to_broadcast(
                                    (128, 2, w)),
                            )
                            nc.scalar.activation(
                                out=p_t.rearrange(
                                    "p (h q) -> p h q", h=2)[:, :, qoff:],
                                in_=sst_v,
                                func=Exp,
                                scale=0.125,
                            )
                        else:
                            nc.scalar.activation(
                                out=p_t, in_=s_ps, func=Exp, scale=0.125,
                            )
                        for hp in range(2):
                            h = 2 * c + hp
                            nc.tensor.matmul(
                                pv[hp][:, qoff:],
                                lhsT=vaug[:, kt, h * 65:(h + 1) * 65],
                                rhs=p_t[:, hp * 512 + qoff:(hp + 1) * 512],
                                start=(kt == 0),
                                stop=(kt == nkt - 1),
                            )
                        it += 1
                        if ready_fill and it % spacing == 0:
                            ready_fill.pop(0)[1]()
                    # a filler between the last PV and the epilogue hides the
                    # ln/exp latency from the PE's broadcast matmul
                    if ready_fill:
                        ready_fill.pop(0)[1]()
                    epilogue(c, qt, pv)
                    if c == 3:
                        for tt in range(qt * 4, qt * 4 + 4):
                            for of in range(2):
                                ready_fill.append(
                                    (None, lambda tt=tt, of=of:
                                     outproj_group(tt, of)))
                # chunk tail: remaining fillers
                for _, f in ready_fill:
                    f()

    if split_waits:
        _split_excess_waits(nc)
    return nc


# ---------------------------------------------------------------------------
# Host side
# ---------------------------------------------------------------------------

_NC_CACHE = {}


def _get_nc(t=T):
    if t not in _NC_CACHE:
        _NC_CACHE[t] = build_nc(t)
    return _NC_CACHE[t]


def make_mask():
    # [tri(128x128) | zeros(128x384)]: band tile at column offset qoff adds
    # the triangular block against S columns qoff:qoff+128 and zero beyond
    k = np.arange(128)[:, None]
    q = np.arange(128)[None, :]
    tri = np.where(k <= q, 0.0, MASK_NEG).astype(np.float32)
    return np.concatenate([tri, np.zeros((128, 384), np.float32)], axis=1)


def core_inputs(x, Wq, bq, Wk, bk, Wv, bv, Wo, core):
    """Build the input map for one core (batch b, head group g)."""
    b, g = divmod(core, 2)
    gs = slice(g * G, (g + 1) * G)
    xt = np.ascontiguousarray(x[b].T).astype(BF16)            # [C, T]
    wqt = np.ascontiguousarray(Wq[gs, :].T).astype(BF16)      # [C, G]
    wkt = np.ascontiguousarray(Wk[gs, :].T).astype(BF16)
    wvt = np.ascontiguousarray(Wv[gs, :].T).astype(BF16)
    wot = np.ascontiguousarray(Wo[:, gs].T).astype(BF16)      # [G, C]
    bqk = np.concatenate(
        [bq[gs].reshape(4, 128).T, bk[gs].reshape(4, 128).T], axis=1
    ).astype(np.float32)                                      # [128, 8]
    return {
        "xt": xt, "wqt": wqt, "wkt": wkt, "wvt": wvt, "wot": wot,
        "bqk": bqk, "bv": bv[gs].astype(np.float32), "mask": make_mask(),
    }


def kernel(x, Wq, bq, Wk, bk, Wv, bv, Wo, bo, _trace=False):
    x = np.asarray(x, dtype=np.float32)
    nc = _get_nc(T)
    in_maps = [
        core_inputs(x, Wq, bq, Wk, bk, Wv, bv, Wo, c) for c in range(N_CORES)
    ]
    res = run_bass_kernel_spmd(nc, in_maps, list(range(N_CORES)), trace=_trace)
    out = np.empty((B, T, C), dtype=np.float32)
    bo = np.asarray(bo, dtype=np.float32)
    for b in range(B):
        out[b] = res.results[2 * b]["out"] + res.results[2 * b + 1]["out"]
        out[b] += bo[None, :]
    kernel.last_results = res
    return out


# revision 25
# speedup vs baseline: 1.1324x; 1.0157x over previous
"""Self-contained Trainium2 (Bass/Tile) multi-head attention kernel.

Problem: nn_MultiHeadAttention (B=4, T=2048, C=1024, H=16 heads, D=64),
fp32, causal, torch-Linear-style projections (y = x @ W.T + b).

Sharding (8 NeuronCores): data-parallel over B (4) x tensor-parallel over
head-groups (2 groups of 8 heads). Core c handles batch c//2, head group
c%2. Each core computes Q/K/V projections for its 512 features, causal
attention for its 8 heads, and a partial output projection
(O_group @ Wo[:, group].T). The host sums the two partials per batch and
adds bo.

On-device layout (per core) is fully "transposed" so that no tensor ever
needs an on-chip transpose:
  - Q^T, K^T: [feat 512, tok T]   (feature-major; head pair per 128-chunk)
  - V_aug:    [tok T, 8*65]       (token-major; per head 64 V cols + ones col)
  - S^T tile: [k 128, q 512] = matmul(lhsT=K^T chunk, rhs=Q^T chunk), K=64
  - P^T = exp(0.125 * S^T + mask) (ScalarE, bf16 out)
  - PV: [65, q 512] += matmul(lhsT=V_aug[k-tile, head], rhs=P^T) over k
        rows 0..63 = unnormalized O^T, row 64 = softmax denominators
  - normalize: r = 1/sums (DVE), broadcast across partitions with a K=1
    matmul (ones^T @ r), multiply on DVE -> O^T [feat, tok] bf16
  - out partial [tok, C] = matmul(lhsT=O^T chunks, rhs=Wo_g^T chunks)
"""

import numpy as np
import ml_dtypes

import bass_rust
import concourse.bass as bass
import concourse.mybir as mybir
import concourse.tile as tile
from concourse.bass_utils import run_bass_kernel_spmd
from concourse.vector_clock import ScopedClock

BF16 = ml_dtypes.bfloat16

B, T, C, H, D = 4, 2048, 1024, 16, 64
G = C // 2          # features per head group (8 heads x 64)
N_CORES = 8
MASK_NEG = -800.0   # pre-scale; exp(0.125 * (s - 800)) == 0 for |s| < 30

# ---------------------------------------------------------------------------
# The walrus build in this container rejects instructions carrying more than
# a couple of sync waits ("Too many sync wait commands"). Tile's kernel-tail
# drain aggregates one wait per live semaphore. Split them into individual
# SP wait instructions (program order on SP preserves the semantics).
# ---------------------------------------------------------------------------


def _patched_drain_and_barrier(self, tick_clock, wait_clock):
    nc = self.nc
    drain_inst = nc.sync.drain()
    wait_clock.add_sem_waits(
        drain_inst.ins, ScopedClock({None: tick_clock.global_clock})
    )
    si = drain_inst.ins.sync_info
    waits = list(si.on_wait) if si is not None else []
    if waits:
        drain_inst.ins.sync_info = bass_rust.SyncInfo(
            on_wait=[], on_update=list(si.on_update)
        )
        assert self.sems is not None
        by_name = {h.name: h for h in self.sems.allocated().values()}
        for w in waits:
            assert w.wait_mode == "sem-ge-imm", w
            nc.sync.wait_ge(by_name[w.ant_name], w.wait_value)

    nc.all_engine_barrier()
    assert self.sems is not None
    popped = nc._tile_sem_poison_stack.pop()
    assert popped is self._sem_poison
    nc.clear_and_free_semaphores(list(self.sems.allocated().values()))
    nc.all_engine_barrier()


tile.TileContext._drain_and_barrier = _patched_drain_and_barrier


def _split_excess_waits(nc, max_waits=1):
    """Hoist surplus sync waits into standalone same-engine EventSemaphore
    instructions placed right before the owner (this walrus encodes at most
    one wait per instruction)."""
    n = 0
    for fn in nc.m.functions:
        for blk in fn.blocks:
            new_insts = []
            for inst in blk.instructions:
                si = inst.sync_info
                waits = list(si.on_wait) if si is not None else []
                if len(waits) > max_waits:
                    for w in waits[:-max_waits]:
                        ev = mybir.InstEventSemaphore(
                            name=f"I-wsplit-{n}", ins=[], outs=[]
                        )
                        n += 1
                        ev.engine = inst.engine
                        ev.sync_info = bass_rust.SyncInfo(
                            on_wait=[w], on_update=[]
                        )
                        new_insts.append(ev)
                    inst.sync_info = bass_rust.SyncInfo(
                        on_wait=waits[-max_waits:], on_update=list(si.on_update)
                    )
                new_insts.append(inst)
            blk.instructions = new_insts


# ---------------------------------------------------------------------------
# Kernel builder (per-core program; same program on all 8 cores)
# ---------------------------------------------------------------------------

def build_nc(t=T, split_waits=True):
    f32 = mybir.dt.float32
    f32r = mybir.dt.float32r
    bf16 = mybir.dt.bfloat16
    Exp = mybir.ActivationFunctionType.Exp

    assert t % 512 == 0
    TS = t // 512            # 512-token slices (q-tiles)
    TK = t // 128            # 128-token k-tiles

    nc = bass.Bass()
    xt_d = nc.dram_tensor("xt", [C, t], bf16, kind="ExternalInput")
    wqt_d = nc.dram_tensor("wqt", [C, G], bf16, kind="ExternalInput")
    wkt_d = nc.dram_tensor("wkt", [C, G], bf16, kind="ExternalInput")
    wvt_d = nc.dram_tensor("wvt", [C, G], bf16, kind="ExternalInput")
    wot_d = nc.dram_tensor("wot", [G, C], bf16, kind="ExternalInput")
    bqk_d = nc.dram_tensor("bqk", [128, 8], f32, kind="ExternalInput")
    bv_d = nc.dram_tensor("bv", [G], f32, kind="ExternalInput")
    mask_d = nc.dram_tensor("mask", [128, 512], f32, kind="ExternalInput")
    out_d = nc.dram_tensor("out", [t, C], f32, kind="ExternalOutput")
    rsc_d = nc.dram_tensor("rscratch", [16, 512], bf16, kind="ExternalOutput")

    with tile.TileContext(nc) as tc:
        with (
            tc.tile_pool(name="big", bufs=1) as big,
            tc.tile_pool(name="weights", bufs=1) as wpool,
            tc.tile_pool(name="xsl", bufs=3) as xpool,
            tc.tile_pool(name="pt", bufs=6) as ptpool,
            tc.tile_pool(name="small", bufs=4) as small,
            tc.tile_pool(name="psA", bufs=2, space="PSUM") as psA,
            tc.tile_pool(name="psS", bufs=2, space="PSUM") as psS,
            tc.tile_pool(name="psPV", bufs=2, space="PSUM") as psPV,
        ):
            # ---- persistent SBUF tensors ----
            qt_sb = big.tile([128, 4, t], bf16, tag="qt")      # Q^T
            kt_sb = big.tile([128, 4, t], bf16, tag="kt")      # K^T
            vaug = big.tile([128, TK, 8 * 65], bf16, tag="va")  # V + ones col
            ot_sb = big.tile([128, 4, t], bf16, tag="ot")      # O^T normalized

            bv_sb = wpool.tile([128, G], f32, tag="bv")
            nc.sync.dma_start(
                out=bv_sb, in_=bv_d[:].unsqueeze(0).to_broadcast((128, G))
            )
            w_sb = {}
            for name, d_t in (("v", wvt_d), ("q", wqt_d), ("k", wkt_d)):
                w = wpool.tile([128, 8, G], bf16, tag=f"w{name}")
                wsrc = d_t[:, :].rearrange("(c p) f -> p c f", p=128)
                nc.sync.dma_start(out=w[:, 0:4, :], in_=wsrc[:, 0:4, :])
                nc.sync.dma_start(out=w[:, 4:8, :], in_=wsrc[:, 4:8, :])
                w_sb[name] = w
            mask_sb = wpool.tile([128, 512], f32, tag="mask")
            nc.sync.dma_start(out=mask_sb, in_=mask_d[:, :])
            bqk_sb = wpool.tile([128, 8], f32, tag="bqk")
            nc.sync.dma_start(out=bqk_sb, in_=bqk_d[:, :])
            ones_sb = wpool.tile([1, 64], bf16, tag="ones")
            nc.vector.memset(ones_sb, 1.0)
            # ones columns of V_aug (head h, column 64)
            va_ones = vaug[:].rearrange("p k (h x) -> p k h x", x=65)[:, :, :, 64]
            nc.vector.memset(va_ones, 1.0)
            wot_sb = wpool.tile([128, 4, C], bf16, tag="wot")
            nc.sync.dma_start(
                out=wot_sb, in_=wot_d[:, :].rearrange("(c p) f -> p c f", p=128)
            )

            def load_x_slice(ts):
                tsl = slice(ts * 512, (ts + 1) * 512)
                x_sl = xpool.tile([128, 8, 512], bf16, tag="xsl")
                src = xt_d[:, tsl].rearrange("(c p) t -> p c t", p=128)
                nc.sync.dma_start(out=x_sl[:, 0:4, :], in_=src[:, 0:4, :])
                nc.sync.dma_start(out=x_sl[:, 4:8, :], in_=src[:, 4:8, :])
                return x_sl

            def v_group(ts):
                """V projection for 4 token sub-tiles: [tok 128, feat 512]
                scattered into vaug (stride 65, per-head ones column kept)."""
                x_sl = load_x_slice(ts)
                for tsub in range(4):
                    kt_idx = ts * 4 + tsub
                    ps = psA.tile([128, 512], f32, tag="mm")
                    for cc in range(8):
                        nc.tensor.matmul(
                            ps,
                            lhsT=x_sl[:, cc, tsub * 128:(tsub + 1) * 128],
                            rhs=w_sb["v"][:, cc, :],
                            start=(cc == 0),
                            stop=(cc == 7),
                        )
                    dst = vaug[:, kt_idx, :].rearrange(
                        "p (h x) -> p h x", x=65)[:, :, 0:64]
                    nc.vector.tensor_add(
                        out=dst,
                        in0=ps.rearrange("p (h d) -> p h d", d=64),
                        in1=bv_sb.rearrange("p (h d) -> p h d", d=64),
                    )

            def qk_group(c, ts):
                """Q^T and K^T projection tiles [feat 128, tok 512] for
                head-pair chunk c, token slice ts."""
                tsl = slice(ts * 512, (ts + 1) * 512)
                x_sl = load_x_slice(ts)
                for name, dst, bcol in (("q", qt_sb, 0), ("k", kt_sb, 4)):
                    ps = psA.tile([128, 512], f32, tag="mm")
                    for cc in range(8):
                        nc.tensor.matmul(
                            ps,
                            lhsT=w_sb[name][:, cc, c * 128:(c + 1) * 128],
                            rhs=x_sl[:, cc, :],
                            start=(cc == 0),
                            stop=(cc == 7),
                        )
                    # copy + per-feature bias (exact, on DVE)
                    with nc.allow_low_precision(
                        reason="Q^T/K^T stored as fp32r for the PE"
                    ):
                        nc.vector.tensor_scalar_add(
                            out=dst[:, c, tsl],
                            in0=ps,
                            scalar1=bqk_sb[:, bcol + c:bcol + c + 1],
                        )

            def outproj_group(tt, of):
                """Partial output projection [tok 128, outfeat 512]."""
                ps = psA.tile([128, 512], f32, tag="mm")
                for fc in range(4):
                    nc.tensor.matmul(
                        ps,
                        lhsT=ot_sb[:, fc, tt * 128:(tt + 1) * 128],
                        rhs=wot_sb[:, fc, of * 512:(of + 1) * 512],
                        start=(fc == 0),
                        stop=(fc == 3),
                    )
                o_out = small.tile([128, 512], f32, tag="oout")
                nc.vector.tensor_copy(out=o_out, in_=ps)
                nc.sync.dma_start(
                    out=out_d[tt * 128:(tt + 1) * 128,
                              of * 512:(of + 1) * 512],
                    in_=o_out,
                )

            # Softmax normalization. Staging the unnormalized O^T rows to
            # SBUF frees the PV banks immediately; r = exp(-ln(sums)) on
            # ScalarE is partition-broadcast by a DRAM-bounce DMA (idle DMA
            # queues; this walrus has no usable on-chip broadcast op) and
            # applied in place on DVE. Nothing here sits on the PE's
            # critical path.
            norm_state = {"nf": 0}

            def stage_epilogue(c, qt, pv):
                for hp in range(2):
                    po = hp * 64
                    lnr = small.tile([1, 512], f32, tag="lnr")
                    nc.scalar.activation(
                        lnr, pv[hp][64:65, :],
                        func=mybir.ActivationFunctionType.Ln,
                    )
                    r_sb = small.tile([1, 512], bf16, tag="r")
                    nc.scalar.activation(r_sb, lnr, func=Exp, scale=-1.0)
                    osl = ot_sb[po:po + 64, c, qt * 512:(qt + 1) * 512]
                    with nc.allow_low_precision(
                        reason="O^T staged bf16; normalized in place"
                    ):
                        nc.vector.tensor_copy(out=osl, in_=pv[hp][0:64, :])
                    slot = norm_state["nf"] % 16
                    norm_state["nf"] += 1
                    nc.sync.dma_start(
                        out=rsc_d[slot:slot + 1, :], in_=r_sb[0:1, :])
                    # both SBUF inputs of a DVE op must share the base
                    # partition: land the broadcast on the head's rows
                    bcast = small.tile([128, 512], bf16, tag="bcast")
                    nc.sync.dma_start(
                        out=bcast[po:po + 64, :],
                        in_=rsc_d[slot, :].unsqueeze(0).to_broadcast(
                            (64, 512)),
                    )
                    nc.vector.tensor_mul(
                        out=osl, in0=osl, in1=bcast[po:po + 64, :])

            def flush_norm(c):
                pass

            # ---- prologue: just enough for (chunk 0, q-tile 0) ----
            v_group(0)
            qk_group(0, 0)

            # ---- attention, with PE filler work interleaved ----
            # Fillers keep TensorE continuously busy through the ACT-bound
            # attention stretches (HAM re-throttles the PE clock to 1.2 GHz
            # after ~3.4us of sub-full activity, halving matmul speed).
            total_iters = sum(4 * (qt + 1) for qt in range(TS))
            for c in range(4):
                # fillers: (deadline_qt, emit_fn). A filler with deadline d
                # MUST be emitted before q-tile d starts (Tile dependencies
                # follow trace order, so a PV reading vaug must come after
                # the V write in emission order). Each chunk carries its own
                # later Q/K slices (deadline = their q-tile) plus the next
                # chunk's first slice, so projections overlap attention
                # maximally.
                fillers = []
                for ts in range(1, TS):
                    if c == 0:
                        fillers.append((ts, lambda ts=ts: v_group(ts)))
                    fillers.append((ts, lambda ts=ts: qk_group(c, ts)))
                if c < 3:
                    fillers.append(
                        (None, lambda cn=c + 1: qk_group(cn, 0)))
                ready_fill = list(fillers)
                spacing = 2 if c == 3 else max(
                    1, total_iters // max(1, len(fillers) + 1))
                it = 0
                for qt in range(TS):
                    while ready_fill and ready_fill[0][0] is not None \
                            and ready_fill[0][0] <= qt:
                        ready_fill.pop(0)[1]()
                    pv = [
                        psPV.tile([65, 512], f32, tag="pv", name=f"pv{i}")
                        for i in range(2)
                    ]
                    nkt = 4 * (qt + 1)
                    for kt in range(nkt):
                        j = kt - 4 * qt      # >=0 on the diagonal band
                        qoff = max(j, 0) * 128      # first valid q column
                        w = 512 - qoff
                        qsl = slice(qt * 512 + qoff, (qt + 1) * 512)
                        # both heads share one 2-bank PSUM tile so the mask
                        # add and the exp run as ONE op each (~250ns fixed
                        # cost per ACT/DVE op otherwise doubles up)
                        s_ps = psS.tile([128, 1024], f32, tag="s")
                        for hp in range(2):
                            po = hp * 64
                            nc.tensor.matmul(
                                s_ps[:, hp * 512 + qoff:(hp + 1) * 512],
                                lhsT=kt_sb[po:po + 64, c,
                                           kt * 128:(kt + 1) * 128],
                                rhs=qt_sb[po:po + 64, c, qsl],
                                start=True,
                                stop=True,
                            )
                        p_t = ptpool.tile([128, 1024], bf16, tag="pt")
                        if j >= 0:
                            s_stage = small.tile([128, 1024], f32, tag="sst")
                            sps_v = s_ps.rearrange(
                                "p (h q) -> p h q", h=2)[:, :, qoff:]
                            sst_v = s_stage.rearrange(
                                "p (h q) -> p h q", h=2)[:, :, :w]
                            nc.vector.tensor_add(
                                out=sst_v,
                                in0=sps_v,
                                in1=mask_sb[:, :w].unsqueeze(1).to_broadcast(
                                    (128, 2, w)),
                            )
                            nc.scalar.activation(
                                out=p_t.rearrange(
                                    "p (h q) -> p h q", h=2)[:, :, qoff:],
                                in_=sst_v,
                                func=Exp,
                                scale=0.125,
                            )
                        else:
                            nc.scalar.activation(
                                out=p_t, in_=s_ps, func=Exp, scale=0.125,
                            )
                        for hp in range(2):
                            h = 2 * c + hp
                            nc.tensor.matmul(
                                pv[hp][:, qoff:],
                                lhsT=vaug[:, kt, h * 65:(h + 1) * 65],
                                rhs=p_t[:, hp * 512 + qoff:(hp + 1) * 512],
                                start=(kt == 0),
                                stop=(kt == nkt - 1),
                            )
                        it += 1
                        if ready_fill and it % spacing == 0:
                            ready_fill.pop(0)[1]()
                    # a filler between the last PV and the epilogue hides the
                    # ln/exp latency from the PE's broadcast matmul
                    if ready_fill:
                        ready_fill.pop(0)[1]()
                    stage_epilogue(c, qt, pv)
                    if c == 3:
                        flush_norm(c)
                        for tt in range(qt * 4, qt * 4 + 4):
                            for of in range(2):
                                ready_fill.append(
                                    (None, lambda tt=tt, of=of:
                                     outproj_group(tt, of)))
                # chunk tail: flush normalizations, remaining fillers
                flush_norm(c)
                for _, f in ready_fill:
                    f()

    if split_waits:
        _split_excess_waits(nc)
    return nc


# ---------------------------------------------------------------------------
# Host side
# ---------------------------------------------------------------------------

_NC_CACHE = {}


def _get_nc(t=T):
    if t not in _NC_CACHE:
        _NC_CACHE[t] = build_nc(t)
    return _NC_CACHE[t]


def make_mask():
    # [tri(128x128) | zeros(128x384)]: band tile at column offset qoff adds
    # the triangular block against S columns qoff:qoff+128 and zero beyond
    k = np.arange(128)[:, None]
    q = np.arange(128)[None, :]
    tri = np.where(k <= q, 0.0, MASK_NEG).astype(np.float32)
    return np.concatenate([tri, np.zeros((128, 384), np.float32)], axis=1)


def core_inputs(x, Wq, bq, Wk, bk, Wv, bv, Wo, core):
    """Build the input map for one core (batch b, head group g)."""
    b, g = divmod(core, 2)
    gs = slice(g * G, (g + 1) * G)
    xt = np.ascontiguousarray(x[b].T).astype(BF16)            # [C, T]
    wqt = np.ascontiguousarray(Wq[gs, :].T).astype(BF16)      # [C, G]
    wkt = np.ascontiguousarray(Wk[gs, :].T).astype(BF16)
    wvt = np.ascontiguousarray(Wv[gs, :].T).astype(BF16)
    wot = np.ascontiguousarray(Wo[:, gs].T).astype(BF16)      # [G, C]
    bqk = np.concatenate(
        [bq[gs].reshape(4, 128).T, bk[gs].reshape(4, 128).T], axis=1
    ).astype(np.float32)                                      # [128, 8]
    return {
        "xt": xt, "wqt": wqt, "wkt": wkt, "wvt": wvt, "wot": wot,
        "bqk": bqk, "bv": bv[gs].astype(np.float32), "mask": make_mask(),
    }


def kernel(x, Wq, bq, Wk, bk, Wv, bv, Wo, bo, _trace=False):
    x = np.asarray(x, dtype=np.float32)
    nc = _get_nc(T)
    in_maps = [
        core_inputs(x, Wq, bq, Wk, bk, Wv, bv, Wo, c) for c in range(N_CORES)
    ]
    res = run_bass_kernel_spmd(nc, in_maps, list(range(N_CORES)), trace=_trace)
    out = np.empty((B, T, C), dtype=np.float32)
    bo = np.asarray(bo, dtype=np.float32)
    for b in range(B):
        out[b] = res.results[2 * b]["out"] + res.results[2 * b + 1]["out"]
        out[b] += bo[None, :]
    kernel.last_results = res
    return out
